# revision 30
# baseline (speedup 1.0000x reference)
"""Trainium2 Bass kernel for nn_Actor (RSNorm -> Linear -> 4x residual LN-MLP
blocks -> post-LN -> clipped mu/std heads), data-parallel over batch on 8
NeuronCores.

Strategy:
- Shard batch B=16384 into 8x2048 rows; weights replicated per core.
- RSNorm (Welford scan over batch) == population mean/var over batch; computed
  via per-shard bn_stats merged across cores with a tiny (4KB) AllReduce.
- All norms are folded into the adjacent matmuls: per-feature affine goes into
  the weight matrix, per-row (mean, std) corrections enter the PSUM
  accumulation as rank-2 matmuls, and the per-row 1/std scale commutes with
  ReLU so it is applied once on the residual update.
- Activations live feature-major ([feat partitions x row free]) so the whole
  residual trunk needs zero transposes; the heads flip back to row-major by
  using the activation tiles as the stationary matmul operand.
- Matmul compute in bf16 (fp32 PSUM accumulate); residual stream stored bf16.
"""

import sys

if "/opt/trn_rl_repo" not in sys.path:
    sys.path.insert(0, "/opt/trn_rl_repo")

import numpy as np

import concourse.bass as bass
import concourse.bacc as bacc
import concourse.mybir as mybir
from concourse import tile
from concourse.bass_utils import run_bass_kernel_spmd

# bass_utils imports antenv.axon_hooks when tracing is requested via
# BASS_TRACE; provide a no-op fallback module when the image lacks it.
try:
    import antenv.axon_hooks  # noqa: F401
except Exception:
    try:
        import types as _types
        import antenv as _antenv

        _m = _types.ModuleType("antenv.axon_hooks")
        _m.get_axon_ntff_profile_hook = lambda: None
        _m.set_axon_ntff_profile_hook = lambda h: None
        _antenv.axon_hooks = _m
        sys.modules["antenv.axon_hooks"] = _m
    except Exception:
        pass

F32 = mybir.dt.float32
BF16 = mybir.dt.bfloat16
AF = mybir.ActivationFunctionType
ALU = mybir.AluOpType

B, DIN, H, A, L = 16384, 512, 1024, 128, 4
NCORES = 8
R = B // NCORES          # 2048 rows per core
CH = 4                   # chunks per core
CW = R // CH             # 512 rows per chunk
KD = DIN // 128          # 4 k-tiles of the input dim
KH = H // 128            # 8 k-tiles of the hidden dim
EPS_RS = 1e-5
EPS_LN = 1e-5

_COMPILED = {}


def _build(fast):
    """fast=True assumes ln_g/post_g == 1 and every bias == 0 (the
    distributions pinned by the problem spec); kernel() verifies before
    dispatching here."""
    nc = bacc.Bacc("TRN2", target_bir_lowering=False, debug=False,
                   num_devices=NCORES)

    stated = nc.dram_tensor("state", [R, DIN], F32, kind="ExternalInput")
    W_ind = nc.dram_tensor("W_in", [DIN, H], F32, kind="ExternalInput")
    b_ind = nc.dram_tensor("b_in", [1, H], F32, kind="ExternalInput")
    ln_gd = nc.dram_tensor("ln_g", [L, H], F32, kind="ExternalInput")
    ln_bd = nc.dram_tensor("ln_b", [L, H], F32, kind="ExternalInput")
    W1d = nc.dram_tensor("W1", [L, H, H], F32, kind="ExternalInput")
    b1d = nc.dram_tensor("b1", [L, H], F32, kind="ExternalInput")
    W2d = nc.dram_tensor("W2", [L, H, H], F32, kind="ExternalInput")
    b2d = nc.dram_tensor("b2", [L, H], F32, kind="ExternalInput")
    post_gd = nc.dram_tensor("post_g", [1, H], F32, kind="ExternalInput")
    post_bd = nc.dram_tensor("post_b", [1, H], F32, kind="ExternalInput")
    Wmud = nc.dram_tensor("Wmu", [H, A], F32, kind="ExternalInput")
    bmud = nc.dram_tensor("bmu", [1, A], F32, kind="ExternalInput")
    Wstdd = nc.dram_tensor("Wstd", [H, A], F32, kind="ExternalInput")
    bstdd = nc.dram_tensor("bstd", [1, A], F32, kind="ExternalInput")
    outd = nc.dram_tensor("out", [R, 2 * A], F32, kind="ExternalOutput")

    identd = nc.inline_tensor(np.eye(128, dtype=np.float32), name="ident")

    # register 1e-5 as a const AP so activation(bias=eps) resolves
    eps_t = nc.alloc_sbuf_tensor("const-eps", [128, 1], F32)
    nc.gpsimd.memset(eps_t.ap(), EPS_LN)
    nc.const_aps.aps[(F32, EPS_LN)] = eps_t.ap()
    nc.all_engine_barrier()

    with tile.TileContext(nc) as tc:
        with (
            tc.tile_pool(name="const", bufs=1) as cp,
            tc.tile_pool(name="xp", bufs=1) as xp,
            tc.tile_pool(name="wp", bufs=2) as wp,
            tc.tile_pool(name="ap", bufs=3) as ap,
            tc.tile_pool(name="rp", bufs=2) as rp,
            tc.tile_pool(name="sp", bufs=4) as sp,
            tc.tile_pool(name="psA", bufs=6, space="PSUM") as psA,
            tc.tile_pool(name="psB", bufs=2, space="PSUM") as psB,
            tc.tile_pool(name="dp", bufs=1, space="DRAM") as dp,
        ):
            # ---------------- constants ----------------
            ones128 = cp.tile([128, 128], BF16)
            nc.vector.memset(ones128[:], 1.0)
            onesrow = cp.tile([1, CW], BF16)
            nc.vector.memset(onesrow[:], 1.0)
            ident = cp.tile([128, 128], F32)
            nc.sync.dma_start(ident[:], identd[:])
            ones128f = cp.tile([128, 128], F32)
            nc.vector.memset(ones128f[:], 1.0)

            if not fast:
                # ln_g/ln_b interleaved column tiles: [128, L*KH*2]
                glb_bf = cp.tile([128, L * KH * 2], BF16)
                glb_f = cp.tile([128, L * KH * 2], F32)
                for src, off in ((ln_gd, 0), (ln_bd, 1)):
                    view = src[:].rearrange("l (k p) -> p (l k)", p=128)
                    dst_bf = glb_bf[:].rearrange("p (lk two) -> p lk two", two=2)
                    dst_f = glb_f[:].rearrange("p (lk two) -> p lk two", two=2)
                    nc.gpsimd.dma_start(dst_bf[:, :, off], view)
                    nc.gpsimd.dma_start(dst_f[:, :, off], view)
                # post_g/post_b column tiles: [128, KH*2]
                pglb_bf = cp.tile([128, KH * 2], BF16)
                for src, off in ((post_gd, 0), (post_bd, 1)):
                    view = src[:].rearrange("o (k p) -> p (o k)", p=128)
                    dst = pglb_bf[:].rearrange("p (k two) -> p k two", two=2)
                    nc.gpsimd.dma_start(dst[:, :, off], view)
                # bias rows
                b_in_row = cp.tile([1, H], F32)
                nc.sync.dma_start(b_in_row[:], b_ind[:])
                bhead = cp.tile([1, 2 * A], F32)
                nc.sync.dma_start(bhead[:, 0:A], bmud[:])
                nc.sync.dma_start(bhead[:, A:2 * A], bstdd[:])

            # ---------------- stage A: state load + transpose ----------------
            xt = [[xp.tile([128, CW], BF16, tag=f"xt_{k}_{c}", name=f"xt_{k}_{c}")
                   for c in range(CH)] for k in range(KD)]
            # state row tiles (bf16) + per-feature batch sums on the PE
            ps_rsum = psA.tile([128, DIN], F32, tag="ps", name="ps_rsum")
            ps_rsq = psA.tile([128, DIN], F32, tag="ps", name="ps_rsq")
            # f32 state rows on the sync queue (4 strips) so the gpsimd
            # queue streams weights concurrently; stats matmuls run in
            # float32r (full rate), transposes in f32 on the PE.
            srows = []
            sstrip = []
            for h in range(4):
                sb = ap.tile([128, 4 * DIN], F32, tag=f"sbig{h % 2}", bufs=1,
                             name=f"sbig{h}")
                nc.sync.dma_start(
                    sb[:].rearrange("p (t c) -> p t c", c=DIN),
                    stated[h * 512:(h + 1) * 512, :].rearrange(
                        "(t p) c -> p t c", p=128))
                sstrip.append(sb)
            for c in range(CH):
                for j in range(4):
                    idx = c * 4 + j
                    srows.append(
                        (c, j,
                         sstrip[idx // 4][:, (idx % 4) * DIN:(idx % 4 + 1) * DIN]))
            for idx, (c, j, srow) in enumerate(srows):
                sqr = ap.tile([128, DIN], F32, tag="sqr", name="sqr")
                nc.scalar.activation(sqr[:], srow, AF.Square)
                nc.tensor.matmul(ps_rsum[:], ones128f[:], srow,
                                 start=(idx == 0), stop=(idx == CH * 4 - 1))
                nc.tensor.matmul(ps_rsq[:], ones128f[:], sqr[:],
                                 start=(idx == 0), stop=(idx == CH * 4 - 1))
            # transpose state tiles on the PE (f32, 2 cyc/row), cast on copy
            for c, j, srow in srows:
                for k in range(KD):
                    pst = psA.tile([128, 128], F32, tag="ps", name="pstr")
                    nc.tensor.transpose(pst[:, :], srow[:, k * 128:(k + 1) * 128],
                                        ident[:])
                    nc.vector.tensor_copy(xt[k][c][:, j * 128:(j + 1) * 128],
                                          pst[:])

            x = [[xp.tile([128, CW], BF16, tag=f"x_{n}_{c}", name=f"x_{n}_{c}")
                  for c in range(CH)] for n in range(KH)]

            # ---------------- helper: per-chunk-pair row stats ----------------
            def stats_chain(pss, psq, eps):
                negm_row = sp.tile([1, CW], BF16, tag="negm", bufs=3,
                                   name="negm_row")
                nc.scalar.activation(negm_row[:], pss[0:1, :], AF.Copy,
                                     scale=-1.0 / H)
                m_b = ap.tile([128, CW], F32, tag="m_b", bufs=2, name="m_b")
                nc.scalar.activation(m_b[:], pss[:], AF.Copy, scale=1.0 / H)
                nc.vector.tensor_tensor(m_b[:], m_b[:], m_b[:], op=ALU.mult)
                var = ap.tile([128, CW], F32, tag="varb", bufs=2, name="var")
                nc.vector.scalar_tensor_tensor(var[:], psq[:], 1.0 / H, m_b[:],
                                               op0=ALU.mult, op1=ALU.subtract)
                q_b = ap.tile([128, CW], F32, tag="qb", bufs=3, name="q_b")
                nc.scalar.activation(q_b[:], var[:], AF.Abs_reciprocal_sqrt,
                                     bias=eps)
                if fast:
                    return q_b, negm_row, None
                vpe = ap.tile([1, CW], F32, tag="vpe", bufs=1, name="vpe")
                nc.vector.tensor_scalar_add(vpe[:], var[0:1, :], eps)
                s_row = sp.tile([1, CW], BF16, tag="s_row", bufs=2,
                                name="s_row")
                nc.vector.tensor_tensor(s_row[:], vpe[:], q_b[0:1, :],
                                        op=ALU.mult)
                return q_b, negm_row, s_row

            def emit_stats_pair(c0, c1, eps):
                pss0 = psA.tile([128, CW], F32, tag="ps", name="pss0")
                psq0 = psA.tile([128, CW], F32, tag="ps", name="psq0")
                pss1 = psA.tile([128, CW], F32, tag="ps", name="pss1")
                psq1 = psA.tile([128, CW], F32, tag="ps", name="psq1")
                for k in range(KH):
                    sq0 = ap.tile([128, CW], BF16, tag="sq", name="sq0")
                    nc.scalar.activation(sq0[:], x[k][c0][:], AF.Square)
                    sq1 = ap.tile([128, CW], BF16, tag="sq", name="sq1")
                    nc.scalar.activation(sq1[:], x[k][c1][:], AF.Square)
                    nc.tensor.matmul(pss0[:], ones128[:], x[k][c0][:],
                                     start=(k == 0), stop=(k == KH - 1))
                    nc.tensor.matmul(psq0[:], ones128[:], sq0[:],
                                     start=(k == 0), stop=(k == KH - 1))
                    nc.tensor.matmul(pss1[:], ones128[:], x[k][c1][:],
                                     start=(k == 0), stop=(k == KH - 1))
                    nc.tensor.matmul(psq1[:], ones128[:], sq1[:],
                                     start=(k == 0), stop=(k == KH - 1))
                st0 = stats_chain(pss0, psq0, eps)
                st1 = stats_chain(pss1, psq1, eps)
                return st0, st1

            # ---------------- block prep (weights + folded rows) ----------------
            def prep_block(l):
                w1big = wp.tile([128, KH * H], BF16, tag="w1big",
                                name=f"w1big_{l}")
                nc.gpsimd.dma_start(
                    w1big[:].rearrange("p (k n) -> p k n", n=H),
                    W1d[l].rearrange("(k p) n -> p k n", p=128))
                w2big = wp.tile([128, KH * H], BF16, tag="w2big",
                                name=f"w2big_{l}")
                nc.gpsimd.dma_start(
                    w2big[:].rearrange("p (k n) -> p k n", n=H),
                    W2d[l].rearrange("(k p) n -> p k n", p=128))
                w1 = [w1big[:, k * H:(k + 1) * H] for k in range(KH)]
                w2 = [w2big[:, k * H:(k + 1) * H] for k in range(KH)]
                w1s_row = sp.tile([1, H], BF16, tag="w1s_row", bufs=2,
                                  name=f"w1s_{l}")
                b2row = cvec_row = None
                if not fast:
                    b1row = sp.tile([1, H], BF16, tag="b1row", bufs=1,
                                    name=f"b1r_{l}")
                    nc.gpsimd.dma_start(b1row[:], b1d[l:l + 1, :])
                    b2row = sp.tile([1, H], BF16, tag="b2row", bufs=1,
                                    name=f"b2r_{l}")
                    nc.gpsimd.dma_start(b2row[:], b2d[l:l + 1, :])
                    cvec_row = sp.tile([1, H], BF16, tag="cvec_row", bufs=1,
                                       name=f"cvec_{l}")
                for half in range(2):
                    psg = psB.tile([1, 512], F32, tag="small", name="psg")
                    for k in range(KH):
                        nc.tensor.matmul(
                            psg[:],
                            ones128[:, 0:1] if fast else
                            glb_bf[:, 2 * (KH * l + k):2 * (KH * l + k) + 1],
                            w1[k][:, half * 512:(half + 1) * 512],
                            start=(k == 0), stop=(k == KH - 1))
                    nc.scalar.activation(w1s_row[0:1, half * 512:(half + 1) * 512],
                                         psg[:], AF.Copy)
                    if not fast:
                        psb_ = psB.tile([1, 512], F32, tag="small", name="psb_")
                        for k in range(KH):
                            nc.tensor.matmul(
                                psb_[:],
                                glb_bf[:, 2 * (KH * l + k) + 1:2 * (KH * l + k) + 2],
                                w1[k][:, half * 512:(half + 1) * 512],
                                start=(k == 0), stop=(k == KH - 1))
                        nc.vector.tensor_tensor(
                            cvec_row[0:1, half * 512:(half + 1) * 512], psb_[:],
                            b1row[:, half * 512:(half + 1) * 512], op=ALU.add)
                if not fast:
                    # W1 <- g * W1 (in place)
                    for k in range(KH):
                        nc.vector.tensor_scalar(
                            w1[k], w1[k],
                            glb_f[:, 2 * (KH * l + k):2 * (KH * l + k) + 1],
                            None, op0=ALU.mult)
                return w1, w2, w1s_row, cvec_row, b2row

            # W_in load, then blocks 0/1 prep DMAs+matmuls, THEN the
            # collective trigger (it parks the gpsimd queue until the
            # all-reduce lands, so every early DMA must be issued first).
            winbig = wp.tile([128, KD * H], BF16, tag="winbig", bufs=1,
                             name="winbig")
            nc.gpsimd.dma_start(
                winbig[:].rearrange("p (k n) -> p k n", n=H),
                W_ind[:].rearrange("(k p) n -> p k n", p=128))
            w_in = [winbig[:, k * H:(k + 1) * H] for k in range(KD)]
            preps = {0: prep_block(0)}
            if fast:
                preps[1] = prep_block(1)

            # ---------------- rsnorm allreduce ----------------
            rsum_row = sp.tile([1, DIN], F32, tag="rsum_row", bufs=1)
            nc.scalar.activation(rsum_row[:], ps_rsum[0:1, :], AF.Copy)
            rsq_row = sp.tile([1, DIN], F32, tag="rsq_row", bufs=1)
            nc.scalar.activation(rsq_row[:], ps_rsq[0:1, :], AF.Copy)
            cc_in = dp.tile([2, DIN], F32)
            cc_out = dp.tile([2, DIN], F32, addr_space="Shared")
            nc.sync.dma_start(cc_in[0:1, :], rsum_row[:])
            nc.sync.dma_start(cc_in[1:2, :], rsq_row[:])
            nc.gpsimd.collective_compute(
                "AllReduce", ALU.add,
                replica_groups=[list(range(NCORES))],
                ins=[cc_in[:].opt()], outs=[cc_out[:].opt()])
            allout = sp.tile([2, DIN], F32, tag="allout", bufs=1)
            nc.sync.dma_start(allout[:], cc_out[:])

            # per-feature fold factors for W_in: transpose [2,128] stat
            # blocks into [128,2] columns, then the scalar chain
            a_col = []
            c_col = []
            for k in range(KD):
                pstc = psB.tile([128, 2], F32, tag="small", name="pstc")
                nc.tensor.transpose(pstc[:],
                                    allout[0:2, k * 128:(k + 1) * 128],
                                    ident[0:2, 0:2])
                stc = sp.tile([128, 2], F32, tag=f"stc_{k}", bufs=1)
                nc.vector.tensor_copy(stc[:], pstc[:])
                muk = sp.tile([128, 1], F32, tag=f"muk_{k}", bufs=1)
                nc.scalar.activation(muk[:], stc[:, 0:1],
                                     AF.Copy, scale=1.0 / B)
                var = sp.tile([128, 1], F32, tag="var1")
                nc.scalar.activation(var[:], stc[:, 1:2],
                                     AF.Copy, scale=1.0 / B)
                msq = sp.tile([128, 1], F32, tag="msq1")
                nc.vector.tensor_tensor(msq[:], muk[:], muk[:], op=ALU.mult)
                nc.vector.tensor_tensor(var[:], var[:], msq[:], op=ALU.subtract)
                nc.vector.tensor_scalar_max(var[:], var[:], 0.001)
                ak = sp.tile([128, 1], F32, tag=f"ak_{k}", bufs=1)
                nc.scalar.activation(ak[:], var[:], AF.Abs_reciprocal_sqrt,
                                     bias=EPS_RS)
                mak = sp.tile([128, 1], F32, tag="mak")
                nc.vector.tensor_tensor(mak[:], muk[:], ak[:], op=ALU.mult)
                ck = sp.tile([128, 1], BF16, tag=f"ck_{k}", bufs=1)
                nc.scalar.activation(ck[:], mak[:], AF.Copy, scale=-1.0)
                a_col.append(ak)
                c_col.append(ck)

            # ---------------- W_in fold (dvec + scale) ----------------
            # dvec = c @ W_in + b_in
            dvec = sp.tile([1, H], BF16, tag="dvec", bufs=1)
            for half in range(2):
                psd = psB.tile([2, 512], F32, tag="small")
                for k in range(KD):
                    nc.tensor.matmul(psd[0:1, :], c_col[k][:],
                                     w_in[k][:, half * 512:(half + 1) * 512],
                                     start=(k == 0), stop=(k == KD - 1))
                if fast:
                    nc.scalar.activation(dvec[:, half * 512:(half + 1) * 512],
                                         psd[0:1, :], AF.Copy)
                else:
                    nc.vector.tensor_tensor(
                        dvec[:, half * 512:(half + 1) * 512], psd[0:1, :],
                        b_in_row[:, half * 512:(half + 1) * 512], op=ALU.add)
            # W_in <- a * W_in (in place, after dvec matmuls)
            for k in range(KD):
                nc.vector.tensor_scalar(w_in[k], w_in[k], a_col[k][:],
                                        None, op0=ALU.mult)



            # ---------------- x1 = folded-rsnorm state @ W_in ----------------
            for c0 in range(0, CH, 2):
                c1 = c0 + 1
                for n in range(KH):
                    ps0 = psA.tile([128, CW], F32, tag="ps", name="ps0")
                    ps1 = psA.tile([128, CW], F32, tag="ps", name="ps1")
                    for k in range(KD):
                        nc.tensor.matmul(ps0[:], w_in[k][:, n * 128:(n + 1) * 128],
                                         xt[k][c0][:], start=(k == 0), stop=False)
                        nc.tensor.matmul(ps1[:], w_in[k][:, n * 128:(n + 1) * 128],
                                         xt[k][c1][:], start=(k == 0), stop=False)
                    nc.tensor.matmul(ps0[:], dvec[:, n * 128:(n + 1) * 128],
                                     onesrow[:], start=False, stop=True)
                    nc.tensor.matmul(ps1[:], dvec[:, n * 128:(n + 1) * 128],
                                     onesrow[:], start=False, stop=True)
                    nc.scalar.activation(x[n][c0][:], ps0[:], AF.Copy)
                    nc.scalar.activation(x[n][c1][:], ps1[:], AF.Copy)

            # ---------------- blocks ----------------
            for l in range(L):
                w1, w2, w1s_row, cvec_row, b2row = (
                    preps[l] if l in preps else prep_block(l))

                for c0 in range(0, CH, 2):
                    c1 = c0 + 1
                    (q0, negm0, srow0), (q1, negm1, srow1) = \
                        emit_stats_pair(c0, c1, EPS_LN)
                    r0_t = []
                    r1_t = []
                    for n in range(KH):
                        psZ0 = psA.tile([128, CW], F32, tag="ps", name="psZ0")
                        psZ1 = psA.tile([128, CW], F32, tag="ps", name="psZ1")
                        for k in range(KH):
                            nc.tensor.matmul(psZ0[:],
                                             w1[k][:, n * 128:(n + 1) * 128],
                                             x[k][c0][:], start=(k == 0),
                                             stop=False)
                            nc.tensor.matmul(psZ1[:],
                                             w1[k][:, n * 128:(n + 1) * 128],
                                             x[k][c1][:], start=(k == 0),
                                             stop=False)
                        nc.tensor.matmul(psZ0[:],
                                         w1s_row[0:1, n * 128:(n + 1) * 128],
                                         negm0[:], start=False, stop=fast)
                        nc.tensor.matmul(psZ1[:],
                                         w1s_row[0:1, n * 128:(n + 1) * 128],
                                         negm1[:], start=False, stop=fast)
                        if not fast:
                            nc.tensor.matmul(
                                psZ0[:], cvec_row[0:1, n * 128:(n + 1) * 128],
                                srow0[:], start=False, stop=True)
                            nc.tensor.matmul(
                                psZ1[:], cvec_row[0:1, n * 128:(n + 1) * 128],
                                srow1[:], start=False, stop=True)
                        r0 = rp.tile([128, CW], BF16, tag=f"r0_{n}", bufs=1,
                                     name=f"r0_{n}")
                        nc.scalar.activation(r0[:], psZ0[:], AF.Relu)
                        r0_t.append(r0)
                        r1 = rp.tile([128, CW], BF16, tag=f"r1_{n}", bufs=1,
                                     name=f"r1_{n}")
                        nc.scalar.activation(r1[:], psZ1[:], AF.Relu)
                        r1_t.append(r1)
                    for n2 in range(KH):
                        psY0 = psA.tile([128, CW], F32, tag="ps", name="psY0")
                        psY1 = psA.tile([128, CW], F32, tag="ps", name="psY1")
                        for n in range(KH):
                            nc.tensor.matmul(psY0[:],
                                             w2[n][:, n2 * 128:(n2 + 1) * 128],
                                             r0_t[n][:], start=(n == 0),
                                             stop=(fast and n == KH - 1))
                            nc.tensor.matmul(psY1[:],
                                             w2[n][:, n2 * 128:(n2 + 1) * 128],
                                             r1_t[n][:], start=(n == 0),
                                             stop=(fast and n == KH - 1))
                        if not fast:
                            nc.tensor.matmul(psY0[:],
                                             b2row[:, n2 * 128:(n2 + 1) * 128],
                                             srow0[:], start=False, stop=True)
                            nc.tensor.matmul(psY1[:],
                                             b2row[:, n2 * 128:(n2 + 1) * 128],
                                             srow1[:], start=False, stop=True)
                        t0 = ap.tile([128, CW], BF16, tag="t", name="t0")
                        nc.vector.tensor_tensor(t0[:], psY0[:], q0[:],
                                                op=ALU.mult)
                        nc.vector.tensor_tensor(x[n2][c0][:], x[n2][c0][:],
                                                t0[:], op=ALU.add)
                        t1 = ap.tile([128, CW], BF16, tag="t", name="t1")
                        nc.vector.tensor_tensor(t1[:], psY1[:], q1[:],
                                                op=ALU.mult)
                        nc.vector.tensor_tensor(x[n2][c1][:], x[n2][c1][:],
                                                t1[:], op=ALU.add)

            # ---------------- heads ----------------
            whbig = wp.tile([128, KH * 2 * A], BF16, tag="whbig", bufs=1,
                            name="whbig")
            whv = whbig[:].rearrange("p (k two a) -> p k two a", two=2, a=A)
            nc.gpsimd.dma_start(whv[:, :, 0, :],
                                Wmud[:].rearrange("(k p) a -> p k a", p=128))
            nc.gpsimd.dma_start(whv[:, :, 1, :],
                                Wstdd[:].rearrange("(k p) a -> p k a", p=128))
            wh = [whbig[:, k * 2 * A:(k + 1) * 2 * A] for k in range(KH)]
            whs_row = sp.tile([1, 2 * A], BF16, tag="whs_row", bufs=1)
            pshg = psB.tile([1, 512], F32, tag="small")
            for k in range(KH):
                nc.tensor.matmul(pshg[:, 0:2 * A],
                                 ones128[:, 0:1] if fast else
                                 pglb_bf[:, 2 * k:2 * k + 1],
                                 wh[k], start=(k == 0), stop=(k == KH - 1))
            nc.scalar.activation(whs_row[:], pshg[:, 0:2 * A], AF.Copy)
            if not fast:
                cvech_row = sp.tile([1, 2 * A], BF16, tag="cvech_row", bufs=1)
                pshb = psB.tile([1, 512], F32, tag="small")
                for k in range(KH):
                    nc.tensor.matmul(pshb[:, 0:2 * A],
                                     pglb_bf[:, 2 * k + 1:2 * k + 2],
                                     wh[k], start=(k == 0), stop=(k == KH - 1))
                nc.vector.tensor_tensor(cvech_row[:], pshb[:, 0:2 * A],
                                        bhead[:], op=ALU.add)

            for c0 in range(0, CH, 2):
                hstats = emit_stats_pair(c0, c0 + 1, EPS_LN)
                for cc in range(2):
                  c = c0 + cc
                  q_b, negm_row, s_row = hstats[cc]
                  for j in range(4):
                    # per-row 1/std as a column: transpose a q_b block
                    pst = psB.tile([128, 128], F32, tag="small", name="pst")
                    nc.tensor.transpose(pst[:], q_b[:, j * 128:(j + 1) * 128],
                                        ident[:])
                    qcol = sp.tile([128, 1], F32, tag="qcol")
                    nc.vector.tensor_copy(qcol[:], pst[:, 0:1])

                    psH = psA.tile([128, 2 * A], F32, tag="ps")
                    for k in range(KH):
                        nc.tensor.matmul(psH[:],
                                         x[k][c][:, j * 128:(j + 1) * 128],
                                         wh[k], start=(k == 0), stop=False)
                    nc.tensor.matmul(psH[:],
                                     negm_row[0:1, j * 128:(j + 1) * 128],
                                     whs_row[:], start=False, stop=fast)
                    if not fast:
                        nc.tensor.matmul(psH[:],
                                         s_row[0:1, j * 128:(j + 1) * 128],
                                         cvech_row[:], start=False, stop=True)
                    outt = ap.tile([128, 2 * A], F32, tag="outt")
                    nc.vector.tensor_scalar(outt[:, 0:A], psH[:, 0:A], qcol[:],
                                            -5.0, op0=ALU.mult, op1=ALU.max)
                    nc.vector.tensor_scalar_min(outt[:, 0:A], outt[:, 0:A], 5.0)
                    nc.vector.tensor_scalar(outt[:, A:2 * A], psH[:, A:2 * A],
                                            qcol[:], 1.0, op0=ALU.mult,
                                            op1=ALU.min)
                    nc.vector.tensor_scalar_max(outt[:, A:2 * A],
                                                outt[:, A:2 * A], -5.0)
                    nc.scalar.activation(outt[:, A:2 * A], outt[:, A:2 * A],
                                         AF.Exp)
                    nc.sync.dma_start(
                        outd[(c * 4 + j) * 128:(c * 4 + j + 1) * 128, :],
                        outt[:])

    nc.compile()
    return nc


def _get_compiled(fast=True):
    if fast not in _COMPILED:
        _COMPILED[fast] = _build(fast)
    return _COMPILED[fast]


def _fast_ok(inputs):
    z = lambda k: not np.any(np.asarray(inputs[k]))
    o = lambda k: np.all(np.asarray(inputs[k]) == 1.0)
    return (z("b_in") and z("ln_b") and z("b1") and z("b2") and z("post_b")
            and z("bmu") and z("bstd") and o("ln_g") and o("post_g"))


def kernel(**inputs):
    nc = _get_compiled(fast=_fast_ok(inputs))
    f = lambda k: np.ascontiguousarray(np.asarray(inputs[k], dtype=np.float32))
    shared = {
        "W_in": f("W_in"),
        "b_in": f("b_in").reshape(1, H),
        "ln_g": f("ln_g"),
        "ln_b": f("ln_b"),
        "W1": f("W1"),
        "b1": f("b1"),
        "W2": f("W2"),
        "b2": f("b2"),
        "post_g": f("post_g").reshape(1, H),
        "post_b": f("post_b").reshape(1, H),
        "Wmu": f("Wmu"),
        "bmu": f("bmu").reshape(1, A),
        "Wstd": f("Wstd"),
        "bstd": f("bstd").reshape(1, A),
    }
    state = f("state")
    in_maps = []
    for i in range(NCORES):
        m = dict(shared)
        m["state"] = state[i * R:(i + 1) * R]
        in_maps.append(m)
    res = run_bass_kernel_spmd(nc, in_maps, core_ids=list(range(NCORES)))
    global LAST_RESULT
    LAST_RESULT = res
    full = np.concatenate([res.results[i]["out"] for i in range(NCORES)], axis=0)
    return full[:, :A].copy(), full[:, A:].copy()


LAST_RESULT = None


# revision 32
# speedup vs baseline: 1.0083x; 1.0083x over previous
"""Trainium2 Bass kernel for nn_Actor (RSNorm -> Linear -> 4x residual LN-MLP
blocks -> post-LN -> clipped mu/std heads), data-parallel over batch on 8
NeuronCores.

Strategy:
- Shard batch B=16384 into 8x2048 rows; weights replicated per core.
- RSNorm (Welford scan over batch) == population mean/var over batch: each
  core reduces its shard with PE ones-matmuls, then a 4KB AllReduce merges
  (sum, sumsq) across cores.
- Every norm is folded into the adjacent matmul: the per-feature affine goes
  into the weight matrix, per-row (mean, std) corrections enter the PSUM
  accumulation as rank-1 matmuls, and the per-row 1/std scale commutes with
  ReLU so it is applied once on the residual update.
- Activations live feature-major ([feat partitions x row free]) so the trunk
  needs zero transposes; the heads flip to row-major by using the activation
  tiles as the stationary matmul operand, so outputs DMA out contiguously.
- Matmul compute in bf16 (fp32 PSUM accumulate); residual stream in bf16.
- The collective trigger parks the gpsimd queue, so the state load runs on
  the sync queue and all early weight DMAs are issued before the trigger;
  W1 loads precede W2 loads so the column-sum prep matmuls of blocks 0-2
  fill the PE while the AllReduce is in flight.
"""

import sys

if "/opt/trn_rl_repo" not in sys.path:
    sys.path.insert(0, "/opt/trn_rl_repo")

import numpy as np

import concourse.bass as bass
import concourse.bacc as bacc
import concourse.mybir as mybir
from concourse import tile
from concourse.bass_utils import run_bass_kernel_spmd

# bass_utils imports antenv.axon_hooks when tracing is requested via
# BASS_TRACE; provide a no-op fallback module when the image lacks it.
try:
    import antenv.axon_hooks  # noqa: F401
except Exception:
    try:
        import types as _types
        import antenv as _antenv

        _m = _types.ModuleType("antenv.axon_hooks")
        _m.get_axon_ntff_profile_hook = lambda: None
        _m.set_axon_ntff_profile_hook = lambda h: None
        _antenv.axon_hooks = _m
        sys.modules["antenv.axon_hooks"] = _m
    except Exception:
        pass

F32 = mybir.dt.float32
BF16 = mybir.dt.bfloat16
AF = mybir.ActivationFunctionType
ALU = mybir.AluOpType

B, DIN, H, A, L = 16384, 512, 1024, 128, 4
NCORES = 8
R = B // NCORES          # 2048 rows per core
CH = 4                   # chunks per core
CW = R // CH             # 512 rows per chunk
KD = DIN // 128          # k-tiles of the input dim
KH = H // 128            # k-tiles of the hidden dim
EPS_RS = 1e-5
EPS_LN = 1e-5

_COMPILED = {}


def _build(fast):
    """fast=True assumes ln_g/post_g == 1 and every bias == 0 (the fills
    pinned by the problem spec); kernel() verifies before dispatching."""
    nc = bacc.Bacc("TRN2", target_bir_lowering=False, debug=False,
                   num_devices=NCORES)

    stated = nc.dram_tensor("state", [R, DIN], F32, kind="ExternalInput")
    W_ind = nc.dram_tensor("W_in", [DIN, H], F32, kind="ExternalInput")
    b_ind = nc.dram_tensor("b_in", [1, H], F32, kind="ExternalInput")
    ln_gd = nc.dram_tensor("ln_g", [L, H], F32, kind="ExternalInput")
    ln_bd = nc.dram_tensor("ln_b", [L, H], F32, kind="ExternalInput")
    W1d = nc.dram_tensor("W1", [L, H, H], F32, kind="ExternalInput")
    b1d = nc.dram_tensor("b1", [L, H], F32, kind="ExternalInput")
    W2d = nc.dram_tensor("W2", [L, H, H], F32, kind="ExternalInput")
    b2d = nc.dram_tensor("b2", [L, H], F32, kind="ExternalInput")
    post_gd = nc.dram_tensor("post_g", [1, H], F32, kind="ExternalInput")
    post_bd = nc.dram_tensor("post_b", [1, H], F32, kind="ExternalInput")
    Wmud = nc.dram_tensor("Wmu", [H, A], F32, kind="ExternalInput")
    bmud = nc.dram_tensor("bmu", [1, A], F32, kind="ExternalInput")
    Wstdd = nc.dram_tensor("Wstd", [H, A], F32, kind="ExternalInput")
    bstdd = nc.dram_tensor("bstd", [1, A], F32, kind="ExternalInput")
    outd = nc.dram_tensor("out", [R, 2 * A], F32, kind="ExternalOutput")

    identd = nc.inline_tensor(np.eye(128, dtype=np.float32), name="ident")

    # register 1e-5 as a const AP so activation(bias=eps) resolves
    eps_t = nc.alloc_sbuf_tensor("const-eps", [128, 1], F32)
    nc.gpsimd.memset(eps_t.ap(), EPS_LN)
    nc.const_aps.aps[(F32, EPS_LN)] = eps_t.ap()
    nc.all_engine_barrier()

    with tile.TileContext(nc) as tc:
        with (
            tc.tile_pool(name="const", bufs=1) as cp,
            tc.tile_pool(name="xp", bufs=1) as xp,
            tc.tile_pool(name="wp", bufs=2) as wp,
            tc.tile_pool(name="ap", bufs=3) as ap,
            tc.tile_pool(name="rp", bufs=2) as rp,
            tc.tile_pool(name="sp", bufs=4) as sp,
            tc.tile_pool(name="psA", bufs=6, space="PSUM") as psA,
            tc.tile_pool(name="psB", bufs=2, space="PSUM") as psB,
            tc.tile_pool(name="dp", bufs=1, space="DRAM") as dp,
        ):
            # ---------------- constants ----------------
            ones128 = cp.tile([128, 128], BF16)
            nc.vector.memset(ones128[:], 1.0)
            onesrow = cp.tile([1, CW], BF16)
            nc.vector.memset(onesrow[:], 1.0)
            ident = cp.tile([128, 128], F32)
            nc.sync.dma_start(ident[:], identd[:])
            if fast:
                ones128f = cp.tile([128, 128], F32)
                nc.vector.memset(ones128f[:], 1.0)
            else:
                identb = cp.tile([128, 128], BF16)
                nc.gpsimd.dma_start(identb[:], identd[:])
                # ln_g/ln_b interleaved column tiles: [128, L*KH*2]
                glb_bf = cp.tile([128, L * KH * 2], BF16)
                glb_f = cp.tile([128, L * KH * 2], F32)
                for src, off in ((ln_gd, 0), (ln_bd, 1)):
                    view = src[:].rearrange("l (k p) -> p (l k)", p=128)
                    dst_bf = glb_bf[:].rearrange("p (lk two) -> p lk two", two=2)
                    dst_f = glb_f[:].rearrange("p (lk two) -> p lk two", two=2)
                    nc.gpsimd.dma_start(dst_bf[:, :, off], view)
                    nc.gpsimd.dma_start(dst_f[:, :, off], view)
                # post_g/post_b column tiles: [128, KH*2]
                pglb_bf = cp.tile([128, KH * 2], BF16)
                for src, off in ((post_gd, 0), (post_bd, 1)):
                    view = src[:].rearrange("o (k p) -> p (o k)", p=128)
                    dst = pglb_bf[:].rearrange("p (k two) -> p k two", two=2)
                    nc.gpsimd.dma_start(dst[:, :, off], view)
                # bias rows
                b_in_row = cp.tile([1, H], F32)
                nc.sync.dma_start(b_in_row[:], b_ind[:])
                bhead = cp.tile([1, 2 * A], F32)
                nc.sync.dma_start(bhead[:, 0:A], bmud[:])
                nc.sync.dma_start(bhead[:, A:2 * A], bstdd[:])

            # residual-stream tiles, feature-major [feat x rows]
            x = [[xp.tile([128, CW], BF16, tag=f"x_{n}_{c}", name=f"x_{n}_{c}")
                  for c in range(CH)] for n in range(KH)]
            # transposed state tiles share the R-tile pool slots (disjoint
            # lifetimes: xt dies at x1, R is born in block 0)
            xt = [[rp.tile([128, CW], BF16, tag=f"r{c % 2}_{k * 2 + c // 2}",
                           bufs=1, name=f"xt_{k}_{c}")
                   for c in range(CH)] for k in range(KD)]

            # ---------------- stage A: state load + shard stats ----------
            ps_rsum = psA.tile([128, DIN], F32, tag="ps", name="ps_rsum")
            ps_rsq = psA.tile([128, DIN], F32, tag="ps", name="ps_rsq")
            srows = []
            if fast:
                # f32 strips on the sync queue; stats matmuls in fp32
                sstrip = []
                for h in range(4):
                    sb = ap.tile([128, 4 * DIN], F32, tag=f"sbig{h % 2}",
                                 bufs=1, name=f"sbig{h}")
                    nc.sync.dma_start(
                        sb[:].rearrange("p (t c) -> p t c", c=DIN),
                        stated[h * 512:(h + 1) * 512, :].rearrange(
                            "(t p) c -> p t c", p=128))
                    sstrip.append(sb)
                for c in range(CH):
                    for j in range(4):
                        idx = c * 4 + j
                        srows.append(
                            (c, j, sstrip[idx // 4]
                             [:, (idx % 4) * DIN:(idx % 4 + 1) * DIN]))
                ones_st = ones128f
                sq_dt = F32
                id_st = ident
            else:
                for c in range(CH):
                    for j in range(4):
                        idx = c * 4 + j
                        srow = ap.tile([128, DIN], BF16, tag=f"srow{idx % 4}",
                                       bufs=1, name=f"srow_{c}_{j}")
                        nc.gpsimd.dma_start(
                            srow[:], stated[idx * 128:(idx + 1) * 128, :])
                        srows.append((c, j, srow[:]))
                ones_st = ones128
                sq_dt = BF16
                id_st = identb
            for idx, (c, j, srow) in enumerate(srows):
                sqr = ap.tile([128, DIN], sq_dt, tag="sqr", name="sqr")
                nc.scalar.activation(sqr[:], srow, AF.Square)
                nc.tensor.matmul(ps_rsum[:], ones_st[:], srow,
                                 start=(idx == 0), stop=(idx == CH * 4 - 1))
                nc.tensor.matmul(ps_rsq[:], ones_st[:], sqr[:],
                                 start=(idx == 0), stop=(idx == CH * 4 - 1))
            # transpose state tiles on the PE, cast to bf16 on the copy out
            for c, j, srow in srows:
                for k in range(KD):
                    pst = psA.tile([128, 128], sq_dt, tag="ps", name="pstr")
                    nc.tensor.transpose(pst[:, :],
                                        srow[:, k * 128:(k + 1) * 128],
                                        id_st[:])
                    nc.vector.tensor_copy(xt[k][c][:, j * 128:(j + 1) * 128],
                                          pst[:])

            # ---------------- early weight DMAs + prep matmuls ------------
            # Everything on the gpsimd queue before the collective trigger.
            winbig = wp.tile([128, KD * H], BF16, tag="winbig", bufs=1,
                             name="winbig")
            nc.gpsimd.dma_start(
                winbig[:].rearrange("p (k n) -> p k n", n=H),
                W_ind[:].rearrange("(k p) n -> p k n", p=128))
            w_in = [winbig[:, k * H:(k + 1) * H] for k in range(KD)]

            w1_bufs = 3 if fast else 2

            def load_w1(l):
                w1big = wp.tile([128, KH * H], BF16, tag="w1big",
                                bufs=w1_bufs, name=f"w1big_{l}")
                nc.gpsimd.dma_start(
                    w1big[:].rearrange("p (k n) -> p k n", n=H),
                    W1d[l].rearrange("(k p) n -> p k n", p=128))
                return [w1big[:, k * H:(k + 1) * H] for k in range(KH)]

            def load_w2(l):
                w2big = wp.tile([128, KH * H], BF16, tag="w2big", bufs=2,
                                name=f"w2big_{l}")
                nc.gpsimd.dma_start(
                    w2big[:].rearrange("p (k n) -> p k n", n=H),
                    W2d[l].rearrange("(k p) n -> p k n", p=128))
                return [w2big[:, k * H:(k + 1) * H] for k in range(KH)]

            def prep_rows(l, w1):
                """Column sums of W1 (and the general-path fold rows)."""
                w1s_row = sp.tile([1, H], BF16, tag="w1s_row",
                                  bufs=4 if fast else 2, name=f"w1s_{l}")
                b2row = cvec_row = None
                if not fast:
                    b1row = sp.tile([1, H], BF16, tag="b1row", bufs=1,
                                    name=f"b1r_{l}")
                    nc.gpsimd.dma_start(b1row[:], b1d[l:l + 1, :])
                    b2row = sp.tile([1, H], BF16, tag="b2row", bufs=1,
                                    name=f"b2r_{l}")
                    nc.gpsimd.dma_start(b2row[:], b2d[l:l + 1, :])
                    cvec_row = sp.tile([1, H], BF16, tag="cvec_row", bufs=1,
                                       name=f"cvec_{l}")
                for half in range(2):
                    psg = psB.tile([1, 512], F32, tag="small", name="psg")
                    for k in range(KH):
                        nc.tensor.matmul(
                            psg[:],
                            ones128[:, 0:1] if fast else
                            glb_bf[:, 2 * (KH * l + k):2 * (KH * l + k) + 1],
                            w1[k][:, half * 512:(half + 1) * 512],
                            start=(k == 0), stop=(k == KH - 1))
                    nc.scalar.activation(
                        w1s_row[0:1, half * 512:(half + 1) * 512], psg[:],
                        AF.Copy)
                    if not fast:
                        psb_ = psB.tile([1, 512], F32, tag="small", name="psb_")
                        for k in range(KH):
                            nc.tensor.matmul(
                                psb_[:],
                                glb_bf[:, 2 * (KH * l + k) + 1:
                                       2 * (KH * l + k) + 2],
                                w1[k][:, half * 512:(half + 1) * 512],
                                start=(k == 0), stop=(k == KH - 1))
                        nc.vector.tensor_tensor(
                            cvec_row[0:1, half * 512:(half + 1) * 512],
                            psb_[:], b1row[:, half * 512:(half + 1) * 512],
                            op=ALU.add)
                if not fast:
                    for k in range(KH):
                        nc.vector.tensor_scalar(
                            w1[k], w1[k],
                            glb_f[:, 2 * (KH * l + k):2 * (KH * l + k) + 1],
                            None, op0=ALU.mult)
                return w1s_row, cvec_row, b2row

            w1_t = {}
            w2_t = {}
            rows_t = {}
            early_w1 = (0, 1, 2) if fast else (0,)
            for l in early_w1:
                w1_t[l] = load_w1(l)
            # heads weights early too (tiny)
            whbig = wp.tile([128, KH * 2 * A], BF16, tag="whbig", bufs=1,
                            name="whbig")
            whv = whbig[:].rearrange("p (k two a) -> p k two a", two=2, a=A)
            nc.gpsimd.dma_start(whv[:, :, 0, :],
                                Wmud[:].rearrange("(k p) a -> p k a", p=128))
            nc.gpsimd.dma_start(whv[:, :, 1, :],
                                Wstdd[:].rearrange("(k p) a -> p k a", p=128))
            wh = [whbig[:, k * 2 * A:(k + 1) * 2 * A] for k in range(KH)]
            for l in ((0, 1) if fast else (0,)):
                w2_t[l] = load_w2(l)
            for l in early_w1:
                rows_t[l] = prep_rows(l, w1_t[l])
            # heads column sums (fill the allreduce window)
            whs_row = sp.tile([1, 2 * A], BF16, tag="whs_row", bufs=1)
            pshg = psB.tile([1, 512], F32, tag="small", name="pshg")
            for k in range(KH):
                nc.tensor.matmul(pshg[:, 0:2 * A],
                                 ones128[:, 0:1] if fast else
                                 pglb_bf[:, 2 * k:2 * k + 1],
                                 wh[k], start=(k == 0), stop=(k == KH - 1))
            nc.scalar.activation(whs_row[:], pshg[:, 0:2 * A], AF.Copy)
            cvech_row = None
            if not fast:
                cvech_row = sp.tile([1, 2 * A], BF16, tag="cvech_row", bufs=1)
                pshb = psB.tile([1, 512], F32, tag="small", name="pshb")
                for k in range(KH):
                    nc.tensor.matmul(pshb[:, 0:2 * A],
                                     pglb_bf[:, 2 * k + 1:2 * k + 2],
                                     wh[k], start=(k == 0), stop=(k == KH - 1))
                nc.vector.tensor_tensor(cvech_row[:], pshb[:, 0:2 * A],
                                        bhead[:], op=ALU.add)

            # ---------------- rsnorm allreduce ----------------
            rsum_row = sp.tile([1, DIN], F32, tag="rsum_row", bufs=1)
            nc.scalar.activation(rsum_row[:], ps_rsum[0:1, :], AF.Copy)
            rsq_row = sp.tile([1, DIN], F32, tag="rsq_row", bufs=1)
            nc.scalar.activation(rsq_row[:], ps_rsq[0:1, :], AF.Copy)
            cc_in = dp.tile([2, DIN], F32)
            cc_out = dp.tile([2, DIN], F32, addr_space="Shared")
            nc.sync.dma_start(cc_in[0:1, :], rsum_row[:])
            nc.sync.dma_start(cc_in[1:2, :], rsq_row[:])
            nc.gpsimd.collective_compute(
                "AllReduce", ALU.add,
                replica_groups=[list(range(NCORES))],
                ins=[cc_in[:].opt()], outs=[cc_out[:].opt()])
            allout = sp.tile([2, DIN], F32, tag="allout", bufs=1)
            nc.sync.dma_start(allout[:], cc_out[:])

            # per-feature fold factors for W_in: transpose [2,128] stat
            # blocks into [128,2] columns, then the scalar chain
            a_col = []
            c_col = []
            for k in range(KD):
                pstc = psB.tile([128, 2], F32, tag="small", name="pstc")
                nc.tensor.transpose(pstc[:],
                                    allout[0:2, k * 128:(k + 1) * 128],
                                    ident[0:2, 0:2])
                stc = sp.tile([128, 2], F32, tag=f"stc_{k}", bufs=1)
                nc.vector.tensor_copy(stc[:], pstc[:])
                muk = sp.tile([128, 1], F32, tag=f"muk_{k}", bufs=1)
                nc.scalar.activation(muk[:], stc[:, 0:1], AF.Copy, scale=1.0 / B)
                var = sp.tile([128, 1], F32, tag="var1")
                nc.scalar.activation(var[:], stc[:, 1:2], AF.Copy, scale=1.0 / B)
                msq = sp.tile([128, 1], F32, tag="msq1")
                nc.vector.tensor_tensor(msq[:], muk[:], muk[:], op=ALU.mult)
                nc.vector.tensor_tensor(var[:], var[:], msq[:], op=ALU.subtract)
                nc.vector.tensor_scalar_max(var[:], var[:], 0.001)
                ak = sp.tile([128, 1], F32, tag=f"ak_{k}", bufs=1)
                nc.scalar.activation(ak[:], var[:], AF.Abs_reciprocal_sqrt,
                                     bias=EPS_RS)
                mak = sp.tile([128, 1], F32, tag="mak")
                nc.vector.tensor_tensor(mak[:], muk[:], ak[:], op=ALU.mult)
                ck = sp.tile([128, 1], BF16, tag=f"ck_{k}", bufs=1)
                nc.scalar.activation(ck[:], mak[:], AF.Copy, scale=-1.0)
                a_col.append(ak)
                c_col.append(ck)

            # ---------------- W_in fold (dvec + scale) ----------------
            dvec = sp.tile([1, H], BF16, tag="dvec", bufs=1)
            for half in range(2):
                psd = psB.tile([2, 512], F32, tag="small")
                for k in range(KD):
                    nc.tensor.matmul(psd[0:1, :], c_col[k][:],
                                     w_in[k][:, half * 512:(half + 1) * 512],
                                     start=(k == 0), stop=(k == KD - 1))
                if fast:
                    nc.scalar.activation(dvec[:, half * 512:(half + 1) * 512],
                                         psd[0:1, :], AF.Copy)
                else:
                    nc.vector.tensor_tensor(
                        dvec[:, half * 512:(half + 1) * 512], psd[0:1, :],
                        b_in_row[:, half * 512:(half + 1) * 512], op=ALU.add)
            # W_in <- a * W_in (in place, after dvec matmuls)
            for k in range(KD):
                nc.vector.tensor_scalar(w_in[k], w_in[k], a_col[k][:],
                                        None, op0=ALU.mult)

            # ---------------- x1 = folded-rsnorm state @ W_in -------------
            for c0 in range(0, CH, 2):
                c1 = c0 + 1
                for n in range(KH):
                    ps0 = psA.tile([128, CW], F32, tag="ps", name="ps0")
                    ps1 = psA.tile([128, CW], F32, tag="ps", name="ps1")
                    for k in range(KD):
                        nc.tensor.matmul(ps0[:],
                                         w_in[k][:, n * 128:(n + 1) * 128],
                                         xt[k][c0][:], start=(k == 0),
                                         stop=False)
                        nc.tensor.matmul(ps1[:],
                                         w_in[k][:, n * 128:(n + 1) * 128],
                                         xt[k][c1][:], start=(k == 0),
                                         stop=False)
                    nc.tensor.matmul(ps0[:], dvec[:, n * 128:(n + 1) * 128],
                                     onesrow[:], start=False, stop=True)
                    nc.tensor.matmul(ps1[:], dvec[:, n * 128:(n + 1) * 128],
                                     onesrow[:], start=False, stop=True)
                    nc.scalar.activation(x[n][c0][:], ps0[:], AF.Copy)
                    nc.scalar.activation(x[n][c1][:], ps1[:], AF.Copy)

            # ---------------- per-chunk-pair row stats ----------------
            def stats_chain(pss, psq, eps):
                negm_row = sp.tile([1, CW], BF16, tag="negm", bufs=3,
                                   name="negm_row")
                nc.scalar.activation(negm_row[:], pss[0:1, :], AF.Copy,
                                     scale=-1.0 / H)
                m_b = ap.tile([128, CW], F32, tag="m_b", bufs=2, name="m_b")
                nc.scalar.activation(m_b[:], pss[:], AF.Copy, scale=1.0 / H)
                nc.vector.tensor_tensor(m_b[:], m_b[:], m_b[:], op=ALU.mult)
                var = ap.tile([128, CW], F32, tag="varb", bufs=2, name="var")
                nc.vector.scalar_tensor_tensor(var[:], psq[:], 1.0 / H, m_b[:],
                                               op0=ALU.mult, op1=ALU.subtract)
                q_b = ap.tile([128, CW], F32, tag="qb", bufs=3, name="q_b")
                nc.scalar.activation(q_b[:], var[:], AF.Abs_reciprocal_sqrt,
                                     bias=eps)
                if fast:
                    return q_b, negm_row, None
                vpe = ap.tile([1, CW], F32, tag="vpe", bufs=1, name="vpe")
                nc.vector.tensor_scalar_add(vpe[:], var[0:1, :], eps)
                s_row = sp.tile([1, CW], BF16, tag="s_row", bufs=2,
                                name="s_row")
                nc.vector.tensor_tensor(s_row[:], vpe[:], q_b[0:1, :],
                                        op=ALU.mult)
                return q_b, negm_row, s_row

            def emit_stats_pair(c0, c1, eps):
                pss0 = psA.tile([128, CW], F32, tag="ps", name="pss0")
                psq0 = psA.tile([128, CW], F32, tag="ps", name="psq0")
                pss1 = psA.tile([128, CW], F32, tag="ps", name="pss1")
                psq1 = psA.tile([128, CW], F32, tag="ps", name="psq1")
                for k in range(KH):
                    sq0 = ap.tile([128, CW], BF16, tag="sq", name="sq0")
                    nc.scalar.activation(sq0[:], x[k][c0][:], AF.Square)
                    sq1 = ap.tile([128, CW], BF16, tag="sq", name="sq1")
                    nc.scalar.activation(sq1[:], x[k][c1][:], AF.Square)
                    nc.tensor.matmul(pss0[:], ones128[:], x[k][c0][:],
                                     start=(k == 0), stop=(k == KH - 1))
                    nc.tensor.matmul(psq0[:], ones128[:], sq0[:],
                                     start=(k == 0), stop=(k == KH - 1))
                    nc.tensor.matmul(pss1[:], ones128[:], x[k][c1][:],
                                     start=(k == 0), stop=(k == KH - 1))
                    nc.tensor.matmul(psq1[:], ones128[:], sq1[:],
                                     start=(k == 0), stop=(k == KH - 1))
                st0 = stats_chain(pss0, psq0, eps)
                st1 = stats_chain(pss1, psq1, eps)
                return st0, st1

            # ---------------- blocks ----------------
            for l in range(L):
                w1 = w1_t[l] if l in w1_t else load_w1(l)
                w2 = w2_t[l] if l in w2_t else load_w2(l)
                if l + 1 < L and l + 1 not in w1_t:
                    w1_t[l + 1] = load_w1(l + 1)
                if l + 1 < L and l + 1 not in w2_t:
                    w2_t[l + 1] = load_w2(l + 1)
                w1s_row, cvec_row, b2row = (
                    rows_t[l] if l in rows_t else prep_rows(l, w1))
                if l + 1 in w1_t and l + 1 not in rows_t:
                    rows_t[l + 1] = prep_rows(l + 1, w1_t[l + 1])

                for c0 in range(0, CH, 2):
                    c1 = c0 + 1
                    (q0, negm0, srow0), (q1, negm1, srow1) = \
                        emit_stats_pair(c0, c1, EPS_LN)
                    r0_t = []
                    r1_t = []
                    for n in range(KH):
                        psZ0 = psA.tile([128, CW], F32, tag="ps", name="psZ0")
                        psZ1 = psA.tile([128, CW], F32, tag="ps", name="psZ1")
                        for k in range(KH):
                            nc.tensor.matmul(psZ0[:],
                                             w1[k][:, n * 128:(n + 1) * 128],
                                             x[k][c0][:], start=(k == 0),
                                             stop=False)
                            nc.tensor.matmul(psZ1[:],
                                             w1[k][:, n * 128:(n + 1) * 128],
                                             x[k][c1][:], start=(k == 0),
                                             stop=False)
                        nc.tensor.matmul(psZ0[:],
                                         w1s_row[0:1, n * 128:(n + 1) * 128],
                                         negm0[:], start=False, stop=fast)
                        nc.tensor.matmul(psZ1[:],
                                         w1s_row[0:1, n * 128:(n + 1) * 128],
                                         negm1[:], start=False, stop=fast)
                        if not fast:
                            nc.tensor.matmul(
                                psZ0[:], cvec_row[0:1, n * 128:(n + 1) * 128],
                                srow0[:], start=False, stop=True)
                            nc.tensor.matmul(
                                psZ1[:], cvec_row[0:1, n * 128:(n + 1) * 128],
                                srow1[:], start=False, stop=True)
                        r0 = rp.tile([128, CW], BF16, tag=f"r0_{n}", bufs=1,
                                     name=f"r0_{n}")
                        nc.scalar.activation(r0[:], psZ0[:], AF.Relu)
                        r0_t.append(r0)
                        r1 = rp.tile([128, CW], BF16, tag=f"r1_{n}", bufs=1,
                                     name=f"r1_{n}")
                        nc.scalar.activation(r1[:], psZ1[:], AF.Relu)
                        r1_t.append(r1)
                    for n2 in range(KH):
                        psY0 = psA.tile([128, CW], F32, tag="ps", name="psY0")
                        psY1 = psA.tile([128, CW], F32, tag="ps", name="psY1")
                        for n in range(KH):
                            nc.tensor.matmul(psY0[:],
                                             w2[n][:, n2 * 128:(n2 + 1) * 128],
                                             r0_t[n][:], start=(n == 0),
                                             stop=(fast and n == KH - 1))
                            nc.tensor.matmul(psY1[:],
                                             w2[n][:, n2 * 128:(n2 + 1) * 128],
                                             r1_t[n][:], start=(n == 0),
                                             stop=(fast and n == KH - 1))
                        if not fast:
                            nc.tensor.matmul(psY0[:],
                                             b2row[:, n2 * 128:(n2 + 1) * 128],
                                             srow0[:], start=False, stop=True)
                            nc.tensor.matmul(psY1[:],
                                             b2row[:, n2 * 128:(n2 + 1) * 128],
                                             srow1[:], start=False, stop=True)
                        t0 = ap.tile([128, CW], BF16, tag="t", name="t0")
                        nc.vector.tensor_tensor(t0[:], psY0[:], q0[:],
                                                op=ALU.mult)
                        nc.vector.tensor_tensor(x[n2][c0][:], x[n2][c0][:],
                                                t0[:], op=ALU.add)
                        t1 = ap.tile([128, CW], BF16, tag="t", name="t1")
                        nc.vector.tensor_tensor(t1[:], psY1[:], q1[:],
                                                op=ALU.mult)
                        nc.vector.tensor_tensor(x[n2][c1][:], x[n2][c1][:],
                                                t1[:], op=ALU.add)

            # ---------------- heads ----------------
            for c0 in range(0, CH, 2):
                hstats = emit_stats_pair(c0, c0 + 1, EPS_LN)
                for cc in range(2):
                    c = c0 + cc
                    q_b, negm_row, s_row = hstats[cc]
                    for j in range(4):
                        # per-row 1/std as a column: transpose a q_b block
                        pst = psB.tile([128, 128], F32, tag="small",
                                       name="pst")
                        nc.tensor.transpose(pst[:],
                                            q_b[:, j * 128:(j + 1) * 128],
                                            ident[:])
                        qcol = sp.tile([128, 1], F32, tag="qcol")
                        nc.vector.tensor_copy(qcol[:], pst[:, 0:1])

                        psH = psA.tile([128, 2 * A], F32, tag="ps", name="psH")
                        for k in range(KH):
                            nc.tensor.matmul(psH[:],
                                             x[k][c][:, j * 128:(j + 1) * 128],
                                             wh[k], start=(k == 0), stop=False)
                        nc.tensor.matmul(psH[:],
                                         negm_row[0:1, j * 128:(j + 1) * 128],
                                         whs_row[:], start=False, stop=fast)
                        if not fast:
                            nc.tensor.matmul(
                                psH[:], s_row[0:1, j * 128:(j + 1) * 128],
                                cvech_row[:], start=False, stop=True)
                        outt = ap.tile([128, 2 * A], F32, tag="outt")
                        nc.vector.tensor_scalar(outt[:, 0:A], psH[:, 0:A],
                                                qcol[:], -5.0, op0=ALU.mult,
                                                op1=ALU.max)
                        nc.vector.tensor_scalar_min(outt[:, 0:A],
                                                    outt[:, 0:A], 5.0)
                        nc.vector.tensor_scalar(outt[:, A:2 * A],
                                                psH[:, A:2 * A], qcol[:], 1.0,
                                                op0=ALU.mult, op1=ALU.min)
                        nc.vector.tensor_scalar_max(outt[:, A:2 * A],
                                                    outt[:, A:2 * A], -5.0)
                        nc.scalar.activation(outt[:, A:2 * A],
                                             outt[:, A:2 * A], AF.Exp)
                        nc.sync.dma_start(
                            outd[(c * 4 + j) * 128:(c * 4 + j + 1) * 128, :],
                            outt[:])

    nc.compile()
    return nc


def _get_compiled(fast=True):
    if fast not in _COMPILED:
        _COMPILED[fast] = _build(fast)
    return _COMPILED[fast]


def _fast_ok(inputs):
    z = lambda k: not np.any(np.asarray(inputs[k]))
    o = lambda k: np.all(np.asarray(inputs[k]) == 1.0)
    return (z("b_in") and z("ln_b") and z("b1") and z("b2") and z("post_b")
            and z("bmu") and z("bstd") and o("ln_g") and o("post_g"))


def kernel(**inputs):
    nc = _get_compiled(fast=_fast_ok(inputs))
    f = lambda k: np.ascontiguousarray(np.asarray(inputs[k], dtype=np.float32))
    shared = {
        "W_in": f("W_in"),
        "b_in": f("b_in").reshape(1, H),
        "ln_g": f("ln_g"),
        "ln_b": f("ln_b"),
        "W1": f("W1"),
        "b1": f("b1"),
        "W2": f("W2"),
        "b2": f("b2"),
        "post_g": f("post_g").reshape(1, H),
        "post_b": f("post_b").reshape(1, H),
        "Wmu": f("Wmu"),
        "bmu": f("bmu").reshape(1, A),
        "Wstd": f("Wstd"),
        "bstd": f("bstd").reshape(1, A),
    }
    state = f("state")
    in_maps = []
    for i in range(NCORES):
        m = dict(shared)
        m["state"] = state[i * R:(i + 1) * R]
        in_maps.append(m)
    res = run_bass_kernel_spmd(nc, in_maps, core_ids=list(range(NCORES)))
    global LAST_RESULT
    LAST_RESULT = res
    full = np.concatenate([res.results[i]["out"] for i in range(NCORES)],
                          axis=0)
    return full[:, :A].copy(), full[:, A:].copy()


LAST_RESULT = None
